# revision 20
# baseline (speedup 1.0000x reference)
"""Trainium2 Bass kernel for CustomMultiHeadAttention (single-query pooled attention).

Reference computation (B=32, S=1024, D=256, H=8):
    keys   = (x @ Wk + bk).reshape(B,S,H,D)
    values = (x @ Wv + bv).reshape(B,S,H,D)
    scores = einsum('bshd,hd->bsh', keys, query)
    attn   = softmax(scores, axis=1)           # over S
    pooled = einsum('bsh,bshd->bhd', attn, values).reshape(B, H*D)
    out    = pooled @ Wo + bo

Algebraic restructure (exact in real arithmetic):
    q_proj[e,h] = sum_d Wk[e, h*D+d] * query[h,d]        # [256, 8]
    scores[b,s,h] = x[b,s,:] @ q_proj[:,h]  (+ const(h) from bk -> cancels in softmax)
    attnu = exp(scores - 64)                             # const shift; softmax invariant
    ctx[b,h,e]  = sum_s attnu[b,s,h] * x[b,s,e];  Z[b,h] = sum_s attnu[b,s,h]
    pooled[b,h,:] = (ctx[b,h,:]/Z[b,h]) @ Wv_h           # sum_s attn = 1
    out = (pooled + bv) @ Wo + bo

This removes both [B*S,256]x[256,2048] projections. The kernel is DMA-bound:
~7.3MB of HBM reads per core against ~13us of PE work and a few us on the
other engines. Implementation notes:
  - All wire tensors are fp16 (halves HBM traffic; 10-bit mantissa keeps the
    values path accurate; attn weights use bf16 for exponent range; all
    matmul accumulation is fp32 in PSUM). The host lays every tensor out
    exactly as it sits in SBUF, so each DMA is one fully contiguous transfer.
  - x is shipped in BOTH orientations ([e,s] for the scores matmul, [s,e]
    (+ two ones columns) for the ctx matmul), in per-batch DMAs that
    pipeline against compute. Measured on HW this beats rebuilding one
    orientation with PE transposes: the PE was the bottleneck, not DMA.
  - q_proj is computed on the PE from a transposed Wk (contract d on
    partitions); Z comes free from the ones columns in the ctx matmul.
  - The output projection runs in transposed orientation (Wo stationary,
    pooled moving); Wo is split into three DMA chunks with a small last
    chunk, so almost nothing trails the last weight byte's arrival. The
    kernel returns out.T per core ([dout, b]); the host transposes during
    the gather.
  - A dummy Exp at the top preloads the activation table off the critical
    path.
Sharding: data-parallel over batch, 4 batches per core on 8 cores.

build_program(loop_n) wraps the whole body in a hardware For_i loop; kernel()
uses loop_n=1. test.py uses larger loop_n to measure true per-iteration HW
time as a wall-clock slope (the ~70ms axon dispatch latency cancels).
"""

import sys

sys.path.insert(0, "/opt/trn_rl_repo")

import numpy as np

import concourse.bass as bass
import concourse.mybir as mybir
import concourse.tile as tile
from concourse import bacc
from concourse.bass_utils import run_bass_kernel_spmd
from concourse.masks import make_identity

F32 = mybir.dt.float32
BF16 = mybir.dt.bfloat16
F16 = mybir.dt.float16
NPF16 = mybir.dt.np(F16)

B, S, D, H = 32, 1024, 256, 8
NCORES = 8
BL = B // NCORES      # local batches per core = 4
ST = S // 128         # s-tiles per batch = 8
KD = 2                # 256 = 2 k-tiles of 128 over the D (input dim) axis
KHD = (H * D) // 128  # 16 k-tiles over the H*D axis
SHIFT = 64.0          # constant score shift before exp (softmax-invariant)
MQT, MBV, MBO = 0, KD * H, KD * H + KHD   # misc tensor column offsets
MISC_COLS = KD * H + KHD + KD             # qt | bvt | boT


def build_program(loop_n=1):
    nc = bacc.Bacc("TRN2", target_bir_lowering=False, debug=False)

    xt_d = nc.dram_tensor("xt", [128, KD, BL, S], F16, kind="ExternalInput")
    xn_d = nc.dram_tensor("xn", [128, BL, ST, D + 2], F16, kind="ExternalInput")
    wkm_d = nc.dram_tensor(
        "wkm", [128, KHD * D + MISC_COLS], F16, kind="ExternalInput"
    )
    wv_d = nc.dram_tensor("wv", [128, KD, H * D], F16, kind="ExternalInput")
    wo_d = nc.dram_tensor("wo", [128, KHD, D], F16, kind="ExternalInput")
    out_d = nc.dram_tensor("out", [128, KD, BL], F32, kind="ExternalOutput")

    with tile.TileContext(nc) as tc:
        with (
            tc.tile_pool(name="big", bufs=1) as big,
            tc.tile_pool(name="sm", bufs=1) as sm,
            tc.tile_pool(name="ps", bufs=1, space=bass.MemorySpace.PSUM) as ps,
            tc.tile_pool(name="pst", bufs=2, space=bass.MemorySpace.PSUM) as pst,
        ):
            # ---- SBUF allocations -------------------------------------
            xt_sb = big.tile([128, KD, BL, S], F16)      # x transposed: p=e%128
            xn_sb = big.tile([128, BL, ST, D + 2], F16)  # x natural + ones cols
            wkm_sb = big.tile([128, KHD * D + MISC_COLS], F16)  # Wk.T | misc
            wv_sb = big.tile([128, KD, H * D], F16)
            wo_sb = big.tile([128, KHD, D], F16)
            wkt_sb = wkm_sb[:, 0:KHD * D].rearrange("p (k d) -> p k d", k=KHD)
            misc_sb = wkm_sb[:, KHD * D:KHD * D + MISC_COLS]
            qp_sb = sm.tile([128, KD, H], F16)           # q_proj [e, h]
            attn_sb = sm.tile([128, BL, ST, H], BF16)    # exp(scores-SHIFT)
            recip = sm.tile([H, BL, 1], F32)             # 1/Z per (h, b)
            ctx_sb = sm.tile([H, BL, D], F16)            # [h, b, e] normalized
            ctxT_sb = sm.tile([128, KD, BL, H], F16)     # [e%128, k, b, h]
            pooledT_sb = sm.tile([128, KHD, BL], F16)    # [(hd)%128, kk, b]
            ident8 = sm.tile([8, 8], F16)
            negs = sm.tile([128, 1], F32)                # -SHIFT bias for exp
            dummy = sm.tile([128, 1], F32)
            outT_sb = sm.tile([128, KD, BL], F32)

            with tc.For_i(0, loop_n):
                # ---- DMA loads (order = transfer order; all contiguous) --
                nc.sync.dma_start(wkm_sb[:], wkm_d[:])
                nc.sync.dma_start(xt_sb[:], xt_d[:])
                nc.sync.dma_start(xn_sb[:], xn_d[:])
                nc.sync.dma_start(wv_sb[:], wv_d[:])
                for lo, hi in ((0, 14), (14, 16)):
                    nc.sync.dma_start(
                        wo_sb[:, lo:hi, :], wo_d[:, lo:hi, :]
                    )

                make_identity(nc, ident8[:])
                nc.vector.memset(negs[:], -SHIFT)
                # dummy exp: pull the act-table load off the critical path
                nc.scalar.activation(
                    dummy[:], negs[:], mybir.ActivationFunctionType.Exp
                )

                # ---- q_proj[e,h] on PE: contract d over partitions -------
                qp_ps = ps.tile([128, KD, H], F32, tag="out")
                for h in range(H):
                    for et in range(KD):
                        for kq in range(KD):
                            nc.tensor.matmul(
                                qp_ps[:, et, h:h + 1],
                                wkt_sb[:, 2 * h + kq, et * 128:(et + 1) * 128],
                                misc_sb[:, MQT + kq * H + h:MQT + kq * H + h + 1],
                                start=(kq == 0),
                                stop=(kq == KD - 1),
                            )
                nc.vector.tensor_copy(qp_sb[:], qp_ps[:])

                # ---- scores[s, h] per (b, t) = xt_tile.T @ q_proj --------
                scores_ps = ps.tile([128, BL, ST, H], F32, tag="scores")
                for b in range(BL):
                    for t in range(ST):
                        for k in range(KD):
                            nc.tensor.matmul(
                                scores_ps[:, b, t, :],
                                xt_sb[:, k, b, t * 128:(t + 1) * 128],
                                qp_sb[:, k, :],
                                start=(k == 0),
                                stop=(k == KD - 1),
                            )
                    nc.scalar.activation(
                        attn_sb[:, b, :, :],
                        scores_ps[:, b, :, :],
                        mybir.ActivationFunctionType.Exp,
                        bias=negs[:],
                    )

                # ---- ctx[h, e] & Z per batch: attnu.T @ [x | 1] ----------
                for b in range(BL):
                    ctx_ps = pst.tile([H, 512], F32, tag="ctx")
                    for t in range(ST):
                        nc.tensor.matmul(
                            ctx_ps[:, 0:D + 2],
                            attn_sb[:, b, t, :],
                            xn_sb[:, b, t, :],
                            start=(t == 0),
                            stop=(t == ST - 1),
                        )
                    nc.vector.reciprocal(recip[:, b, :], ctx_ps[:, D:D + 1])
                    nc.vector.tensor_scalar_mul(
                        ctx_sb[:, b, :],
                        ctx_ps[:, 0:D],
                        recip[:, b, :],
                    )
                    # ctxT[e, (b,h)] via PE transpose
                    for et in range(KD):
                        ctp = pst.tile([128, H], F16, tag="tp")
                        nc.tensor.transpose(
                            ctp[:],
                            ctx_sb[:, b, et * 128:(et + 1) * 128],
                            ident8[:],
                        )
                        nc.vector.tensor_copy(ctxT_sb[:, et, b, :], ctp[:])

                # ---- pooledT[(h d), b] = Wv_h.T @ ctx_h.T ----------------
                pooledT_ps = pst.tile([128, KHD, BL], F32, tag="tp")
                for h in range(H):
                    for dh in range(2):
                        for k in range(KD):
                            nc.tensor.matmul(
                                pooledT_ps[:, h * 2 + dh, :],
                                wv_sb[:, k, h * D + dh * 128: h * D + (dh + 1) * 128],
                                ctxT_sb[:, k, :, h],
                                start=(k == 0),
                                stop=(k == KD - 1),
                            )
                # fold bv in before the output projection: (pooled+bv) @ Wo
                nc.vector.tensor_add(
                    pooledT_sb[:],
                    pooledT_ps[:],
                    misc_sb[:, MBV:MBV + KHD]
                    .rearrange("p k -> p k ()").broadcast_to([128, KHD, BL]),
                )

                # ---- outT[dout, b] = Wo.T-tiles (stationary) x pooledT ---
                # two PSUM tiles (separate banks) so both 128-row output
                # halves accumulate kk-outer, chasing the wo DMA chunks
                oT0_ps = ps.tile([128, BL], F32, tag="out")
                oT1_ps = ps.tile([128, BL], F32, tag="scores")
                oT_ps = [oT0_ps, oT1_ps]
                for kk in range(KHD):
                    for et in range(KD):
                        nc.tensor.matmul(
                            oT_ps[et][:],
                            wo_sb[:, kk, et * 128:(et + 1) * 128],
                            pooledT_sb[:, kk, :],
                            start=(kk == 0),
                            stop=(kk == KHD - 1),
                        )
                # + bo (transposed layout), f32 out
                for et in range(KD):
                    nc.vector.tensor_add(
                        outT_sb[:, et, :],
                        oT_ps[et][:],
                        misc_sb[:, MBO + et:MBO + et + 1]
                        .broadcast_to([128, BL]),
                    )
                nc.sync.dma_start(out_d[:], outT_sb[:])

    nc.compile()
    return nc


_NC_CACHE = {}


def get_nc(loop_n=1):
    if loop_n not in _NC_CACHE:
        _NC_CACHE[loop_n] = build_program(loop_n)
    return _NC_CACHE[loop_n]


def make_in_maps(x, Wk, bk, Wv, bv, query, Wo, bo):
    x = np.ascontiguousarray(x, dtype=np.float32)
    # weight-side wire tensors (shared across cores)
    wkt = np.ascontiguousarray(
        np.asarray(Wk, np.float32).T.astype(NPF16)
        .reshape(KHD, 128, D).transpose(1, 0, 2)
    )
    wv = np.ascontiguousarray(
        np.asarray(Wv, np.float32).astype(NPF16)
        .reshape(KD, 128, H * D).transpose(1, 0, 2)
    )
    wo = np.ascontiguousarray(
        np.asarray(Wo, np.float32).astype(NPF16)
        .reshape(KHD, 128, D).transpose(1, 0, 2)
    )
    wkm = np.zeros((128, KHD * D + MISC_COLS), NPF16)
    wkm[:, 0:KHD * D] = wkt.reshape(128, KHD * D)
    moff = KHD * D
    wkm[:, moff + MQT:moff + MQT + KD * H] = (
        np.asarray(query, np.float32).T.astype(NPF16).reshape(KD, 128, H)
        .transpose(1, 0, 2).reshape(128, KD * H)
    )
    wkm[:, moff + MBV:moff + MBV + KHD] = (
        np.asarray(bv, np.float32).astype(NPF16).reshape(KHD, 128).T
    )
    wkm[:, moff + MBO:moff + MBO + KD] = (
        np.asarray(bo, np.float32).astype(NPF16).reshape(KD, 128).T
    )
    xbf = x.astype(NPF16)
    in_maps = []
    for c in range(NCORES):
        xs = xbf[c * BL:(c + 1) * BL]                      # [BL, S, D]
        xt = np.ascontiguousarray(
            xs.transpose(2, 0, 1).reshape(KD, 128, BL, S).transpose(1, 0, 2, 3)
        )
        # natural layout + two all-ones columns, partition = s%128
        xn1 = np.concatenate(
            [xs, np.ones((BL, S, 2), NPF16)], axis=2
        )                                                  # [BL, S, D+2]
        xn = np.ascontiguousarray(
            xn1.reshape(BL, ST, 128, D + 2).transpose(2, 0, 1, 3)
        )
        in_maps.append({"xt": xt, "xn": xn, "wkm": wkm, "wv": wv, "wo": wo})
    return in_maps


def kernel(x, Wk, bk, Wv, bv, query, Wo, bo):
    nc = get_nc()
    in_maps = make_in_maps(x, Wk, bk, Wv, bv, query, Wo, bo)
    res = run_bass_kernel_spmd(nc, in_maps, core_ids=list(range(NCORES)))
    # per-core output is outT [128, KD, BL]; untranspose to [BL, D]
    return np.concatenate(
        [
            res.results[c]["out"].transpose(2, 1, 0).reshape(BL, D)
            for c in range(NCORES)
        ],
        axis=0,
    )


# revision 24
# speedup vs baseline: 1.0311x; 1.0311x over previous
"""Trainium2 Bass kernel for CustomMultiHeadAttention (single-query pooled attention).

Reference computation (B=32, S=1024, D=256, H=8):
    keys   = (x @ Wk + bk).reshape(B,S,H,D)
    values = (x @ Wv + bv).reshape(B,S,H,D)
    scores = einsum('bshd,hd->bsh', keys, query)
    attn   = softmax(scores, axis=1)           # over S
    pooled = einsum('bsh,bshd->bhd', attn, values).reshape(B, H*D)
    out    = pooled @ Wo + bo

Algebraic restructure (exact in real arithmetic):
    q_proj[e,h] = sum_d Wk[e, h*D+d] * query[h,d]        # [256, 8]
    scores[b,s,h] = x[b,s,:] @ q_proj[:,h]  (+ const(h) from bk -> cancels in softmax)
    attnu = exp(scores - 64)                             # const shift; softmax invariant
    ctx[b,h,e]  = sum_s attnu[b,s,h] * x[b,s,e];  Z[b,h] = sum_s attnu[b,s,h]
    pooled[b,h,:] = (ctx[b,h,:]/Z[b,h]) @ Wv_h           # sum_s attn = 1
    out = (pooled + bv) @ Wo + bo

This removes both [B*S,256]x[256,2048] projections. The kernel is DMA-bound:
~7.3MB of HBM reads per core against ~13us of PE work and a few us on the
other engines. Implementation notes:
  - All wire tensors are fp16 (halves HBM traffic; 10-bit mantissa keeps the
    values path accurate; attn weights use bf16 for exponent range; all
    matmul accumulation is fp32 in PSUM). The host lays every tensor out
    exactly as it sits in SBUF, so each DMA is one fully contiguous transfer.
  - x is shipped in BOTH orientations ([e,s] for the scores matmul, [s,e]
    (+ two ones columns) for the ctx matmul), each as one merged DMA.
    Measured on HW this beats rebuilding one orientation with PE
    transposes (31.0us vs 34.8us per iteration): the PE, including
    LD_WEIGHTS time the cost model omits, was the bottleneck, not DMA.
  - q_proj is computed on the PE from a transposed Wk (contract d on
    partitions); Z comes free from the ones columns in the ctx matmul.
  - The output projection runs in transposed orientation (Wo stationary,
    pooled moving); Wo is split into three DMA chunks with a small last
    chunk, so almost nothing trails the last weight byte's arrival. The
    kernel returns out.T per core ([dout, b]); the host transposes during
    the gather.
  - A dummy Exp at the top preloads the activation table off the critical
    path.
Sharding: data-parallel over batch, 4 batches per core on 8 cores.

build_program(loop_n) wraps the whole body in a hardware For_i loop; kernel()
uses loop_n=1. test.py uses larger loop_n to measure true per-iteration HW
time as a wall-clock slope (the ~70ms axon dispatch latency cancels).
"""

import sys

sys.path.insert(0, "/opt/trn_rl_repo")

import numpy as np

import concourse.bass as bass
import concourse.mybir as mybir
import concourse.tile as tile
from concourse import bacc
from concourse.bass_utils import run_bass_kernel_spmd
from concourse.masks import make_identity

F32 = mybir.dt.float32
BF16 = mybir.dt.bfloat16
F16 = mybir.dt.float16
NPF16 = mybir.dt.np(F16)

B, S, D, H = 32, 1024, 256, 8
NCORES = 8
BL = B // NCORES      # local batches per core = 4
ST = S // 128         # s-tiles per batch = 8
KD = 2                # 256 = 2 k-tiles of 128 over the D (input dim) axis
KHD = (H * D) // 128  # 16 k-tiles over the H*D axis
SHIFT = 64.0          # constant score shift before exp (softmax-invariant)
MQT, MBV, MBO = 0, KD * H, KD * H + KHD   # misc tensor column offsets
MISC_COLS = KD * H + KHD + KD             # qt | bvt | boT


def build_program(loop_n=1):
    nc = bacc.Bacc("TRN2", target_bir_lowering=False, debug=False)

    xt_d = nc.dram_tensor("xt", [128, KD, BL, S], F16, kind="ExternalInput")
    xn_d = nc.dram_tensor("xn", [128, BL, ST, D + 2], F16, kind="ExternalInput")
    wkm_d = nc.dram_tensor(
        "wkm", [128, KHD * D + MISC_COLS], F16, kind="ExternalInput"
    )
    wv_d = nc.dram_tensor("wv", [128, KD, H * D], F16, kind="ExternalInput")
    wo_d = nc.dram_tensor("wo", [128, KHD, D], F16, kind="ExternalInput")
    out_d = nc.dram_tensor("out", [128, KD, BL], F32, kind="ExternalOutput")

    with tile.TileContext(nc) as tc:
        with (
            tc.tile_pool(name="big", bufs=1) as big,
            tc.tile_pool(name="sm", bufs=1) as sm,
            tc.tile_pool(name="ps", bufs=1, space=bass.MemorySpace.PSUM) as ps,
            tc.tile_pool(name="pst", bufs=2, space=bass.MemorySpace.PSUM) as pst,
            tc.tile_pool(name="ctxp", bufs=1, space=bass.MemorySpace.PSUM) as ctxp,
        ):
            # ---- SBUF allocations -------------------------------------
            xt_sb = big.tile([128, KD, BL, S], F16)      # x transposed: p=e%128
            xn_sb = big.tile([128, BL, ST, D + 2], F16)  # x natural + ones cols
            wkm_sb = big.tile([128, KHD * D + MISC_COLS], F16)  # Wk.T | misc
            wv_sb = big.tile([128, KD, H * D], F16)
            wo_sb = big.tile([128, KHD, D], F16)
            wkt_sb = wkm_sb[:, 0:KHD * D].rearrange("p (k d) -> p k d", k=KHD)
            misc_sb = wkm_sb[:, KHD * D:KHD * D + MISC_COLS]
            qp_sb = sm.tile([128, KD, H], F16)           # q_proj [e, h]
            attn_sb = sm.tile([128, BL, ST, H], BF16)    # exp(scores-SHIFT)
            recip = sm.tile([H, BL, 1], F32)             # 1/Z per (h, b)
            ctx_sb = sm.tile([H, BL, D], F16)            # [h, b, e] normalized
            ctxT_sb = sm.tile([128, KD, BL, H], F16)     # [e%128, k, b, h]
            pooledT_sb = sm.tile([128, KHD, BL], F16)    # [(hd)%128, kk, b]
            ident8 = sm.tile([8, 8], F16)
            negs = sm.tile([128, 1], F32)                # -SHIFT bias for exp
            dummy = sm.tile([128, 1], F32)
            outT_sb = sm.tile([128, KD, BL], F32)

            with tc.For_i(0, loop_n):
                # ---- DMA loads (order = transfer order; all contiguous) --
                nc.sync.dma_start(wkm_sb[:], wkm_d[:])
                nc.sync.dma_start(xt_sb[:], xt_d[:])
                nc.sync.dma_start(xn_sb[:], xn_d[:])
                nc.sync.dma_start(wv_sb[:], wv_d[:])
                for lo, hi in ((0, 14), (14, 16)):
                    nc.sync.dma_start(
                        wo_sb[:, lo:hi, :], wo_d[:, lo:hi, :]
                    )

                make_identity(nc, ident8[:])
                nc.vector.memset(negs[:], -SHIFT)
                # dummy exp: pull the act-table load off the critical path
                nc.scalar.activation(
                    dummy[:], negs[:], mybir.ActivationFunctionType.Exp
                )

                # ---- q_proj[e,h] on PE: contract d over partitions -------
                qp_ps = ps.tile([128, KD, H], F32, tag="out")
                for h in range(H):
                    for et in range(KD):
                        for kq in range(KD):
                            nc.tensor.matmul(
                                qp_ps[:, et, h:h + 1],
                                wkt_sb[:, 2 * h + kq, et * 128:(et + 1) * 128],
                                misc_sb[:, MQT + kq * H + h:MQT + kq * H + h + 1],
                                start=(kq == 0),
                                stop=(kq == KD - 1),
                            )
                nc.vector.tensor_copy(qp_sb[:], qp_ps[:])

                # ---- scores[s, h] per (b, t) = xt_tile.T @ q_proj --------
                scores_ps = ps.tile([128, BL, ST, H], F32, tag="scores")
                for b in range(BL):
                    for t in range(ST):
                        for k in range(KD):
                            nc.tensor.matmul(
                                scores_ps[:, b, t, :],
                                xt_sb[:, k, b, t * 128:(t + 1) * 128],
                                qp_sb[:, k, :],
                                start=(k == 0),
                                stop=(k == KD - 1),
                            )
                    nc.scalar.activation(
                        attn_sb[:, b, :, :],
                        scores_ps[:, b, :, :],
                        mybir.ActivationFunctionType.Exp,
                        bias=negs[:],
                    )

                # ---- ctx[h, e] & Z per batch: attnu.T @ [x | 1] ----------
                # one PSUM bank per batch, t-outer emission: four
                # independent accumulation chains pipeline on the PE
                # instead of stalling on chained PSUM read-modify-writes
                ctx_tiles = []
                for b in range(BL):
                    ctx_ps = ctxp.tile([H, 512], F32, tag=f"ctx{b}")
                    ctx_tiles.append(ctx_ps)
                for t in range(ST):
                    for b in range(BL):
                        nc.tensor.matmul(
                            ctx_tiles[b][:, 0:D + 2],
                            attn_sb[:, b, t, :],
                            xn_sb[:, b, t, :],
                            start=(t == 0),
                            stop=(t == ST - 1),
                        )
                for b in range(BL):
                    ctx_ps = ctx_tiles[b]
                    nc.vector.reciprocal(recip[:, b, :], ctx_ps[:, D:D + 1])
                    nc.vector.tensor_scalar_mul(
                        ctx_sb[:, b, :],
                        ctx_ps[:, 0:D],
                        recip[:, b, :],
                    )
                    # ctxT[e, (b,h)] via PE transpose
                    for et in range(KD):
                        ctp = pst.tile([128, H], F16, tag="tp")
                        nc.tensor.transpose(
                            ctp[:],
                            ctx_sb[:, b, et * 128:(et + 1) * 128],
                            ident8[:],
                        )
                        nc.vector.tensor_copy(ctxT_sb[:, et, b, :], ctp[:])

                # ---- pooledT[(h d), b] = Wv_h.T @ ctx_h.T ----------------
                pooledT_ps = pst.tile([128, KHD, BL], F32, tag="tp")
                for h in range(H):
                    for dh in range(2):
                        for k in range(KD):
                            nc.tensor.matmul(
                                pooledT_ps[:, h * 2 + dh, :],
                                wv_sb[:, k, h * D + dh * 128: h * D + (dh + 1) * 128],
                                ctxT_sb[:, k, :, h],
                                start=(k == 0),
                                stop=(k == KD - 1),
                            )
                # fold bv in before the output projection: (pooled+bv) @ Wo
                nc.vector.tensor_add(
                    pooledT_sb[:],
                    pooledT_ps[:],
                    misc_sb[:, MBV:MBV + KHD]
                    .rearrange("p k -> p k ()").broadcast_to([128, KHD, BL]),
                )

                # ---- outT[dout, b] = Wo.T-tiles (stationary) x pooledT ---
                # two PSUM tiles (separate banks) so both 128-row output
                # halves accumulate kk-outer, chasing the wo DMA chunks
                oT0_ps = ps.tile([128, BL], F32, tag="out")
                oT1_ps = ps.tile([128, BL], F32, tag="scores")
                oT_ps = [oT0_ps, oT1_ps]
                for kk in range(KHD):
                    for et in range(KD):
                        nc.tensor.matmul(
                            oT_ps[et][:],
                            wo_sb[:, kk, et * 128:(et + 1) * 128],
                            pooledT_sb[:, kk, :],
                            start=(kk == 0),
                            stop=(kk == KHD - 1),
                        )
                # + bo (transposed layout), f32 out
                for et in range(KD):
                    nc.vector.tensor_add(
                        outT_sb[:, et, :],
                        oT_ps[et][:],
                        misc_sb[:, MBO + et:MBO + et + 1]
                        .broadcast_to([128, BL]),
                    )
                nc.sync.dma_start(out_d[:], outT_sb[:])

    nc.compile()
    return nc


_NC_CACHE = {}


def get_nc(loop_n=1):
    if loop_n not in _NC_CACHE:
        _NC_CACHE[loop_n] = build_program(loop_n)
    return _NC_CACHE[loop_n]


def make_in_maps(x, Wk, bk, Wv, bv, query, Wo, bo):
    x = np.ascontiguousarray(x, dtype=np.float32)
    # weight-side wire tensors (shared across cores)
    wkt = np.ascontiguousarray(
        np.asarray(Wk, np.float32).T.astype(NPF16)
        .reshape(KHD, 128, D).transpose(1, 0, 2)
    )
    wv = np.ascontiguousarray(
        np.asarray(Wv, np.float32).astype(NPF16)
        .reshape(KD, 128, H * D).transpose(1, 0, 2)
    )
    wo = np.ascontiguousarray(
        np.asarray(Wo, np.float32).astype(NPF16)
        .reshape(KHD, 128, D).transpose(1, 0, 2)
    )
    wkm = np.zeros((128, KHD * D + MISC_COLS), NPF16)
    wkm[:, 0:KHD * D] = wkt.reshape(128, KHD * D)
    moff = KHD * D
    wkm[:, moff + MQT:moff + MQT + KD * H] = (
        np.asarray(query, np.float32).T.astype(NPF16).reshape(KD, 128, H)
        .transpose(1, 0, 2).reshape(128, KD * H)
    )
    wkm[:, moff + MBV:moff + MBV + KHD] = (
        np.asarray(bv, np.float32).astype(NPF16).reshape(KHD, 128).T
    )
    wkm[:, moff + MBO:moff + MBO + KD] = (
        np.asarray(bo, np.float32).astype(NPF16).reshape(KD, 128).T
    )
    xbf = x.astype(NPF16)
    in_maps = []
    for c in range(NCORES):
        xs = xbf[c * BL:(c + 1) * BL]                      # [BL, S, D]
        xt = np.ascontiguousarray(
            xs.transpose(2, 0, 1).reshape(KD, 128, BL, S).transpose(1, 0, 2, 3)
        )
        # natural layout + two all-ones columns, partition = s%128
        xn1 = np.concatenate(
            [xs, np.ones((BL, S, 2), NPF16)], axis=2
        )                                                  # [BL, S, D+2]
        xn = np.ascontiguousarray(
            xn1.reshape(BL, ST, 128, D + 2).transpose(2, 0, 1, 3)
        )
        in_maps.append({"xt": xt, "xn": xn, "wkm": wkm, "wv": wv, "wo": wo})
    return in_maps


def kernel(x, Wk, bk, Wv, bv, query, Wo, bo):
    nc = get_nc()
    in_maps = make_in_maps(x, Wk, bk, Wv, bv, query, Wo, bo)
    res = run_bass_kernel_spmd(nc, in_maps, core_ids=list(range(NCORES)))
    # per-core output is outT [128, KD, BL]; untranspose to [BL, D]
    return np.concatenate(
        [
            res.results[c]["out"].transpose(2, 1, 0).reshape(BL, D)
            for c in range(NCORES)
        ],
        axis=0,
    )
